# revision 36
# baseline (speedup 1.0000x reference)
"""Capsule-routing kernel v3 — batch-merged, parity-packed, DMA/engine optimized.

Per core: 8 batches in 2 groups of 4, processed in lockstep so that matmuls
merge across batches and vector/scalar ops run at full [128, *] width.

Index conventions (per group of GB=4 batches):
  capsule n = 2*kc + parity   (kc in [0,16), parity in {0,1})
  slot(b, kc) = b*16 + kc     in [0, 64)
  p'(b, n)  = parity*64 + slot  -> z column order (parity-major)
  b/c layout: [128 (parity*64+G), 32 g, 64 slot]
  o layout:   [64 slot, 2 parity, 64 d]   (enables single out-DMA per group)
Row-packed MM pairs: even capsules use partitions 0:64, odd 64:128, running
concurrently in distinct PE quadrants (bf16).

v3 changes vs v2:
  - DMA: single-copy X^T + on-chip dup, per-group bulk loads, 4 issue queues,
    priority ordering (first matmul ~3us instead of ~17us), 1 out-DMA/group.
  - zstep: one MM per g via duplicated-free-dim stationary wt2 (32 vs 64 MMs).
  - mm2: one MM per g via block-diagonal W (32 vs 64 MMs), new o layout.
  - softmax: full-width fold matrix e2f (drops the broadcast matmul), exp
    reads PSUM directly (overlaps the b-coefficient copy), bf16 exp/cmul,
    kc-reduction on gpsimd.
"""

import numpy as np

B, IN_CAPS, IN_DIM = 64, 2048, 64
NUM, DIM = 32, 64
N_CORES = 8
BPC = B // N_CORES  # 8 batches per core
GB = 4              # batches per merged group
NG = BPC // GB      # 2 groups
EPS = 1e-7

_CACHE = {}


def _build_nc(bpc=BPC):
    import concourse.bacc as bacc
    import concourse.tile as tile
    from concourse import mybir

    f32 = mybir.dt.float32
    bf16 = mybir.dt.bfloat16
    Act = mybir.ActivationFunctionType
    Alu = mybir.AluOpType

    ng = bpc // GB
    nc = bacc.Bacc("TRN2", target_bir_lowering=False, debug=False, num_devices=N_CORES)

    # ---- DRAM I/O (per-core shapes) ----
    # x[b, parity*64+G, kc, i] = X[b, (2kc+parity)*64+G, i]
    x_d = nc.dram_tensor("x", [bpc, 128, 16, IN_DIM], bf16, kind="ExternalInput")
    # xt[b, k, r] = X[b, r, k]   (single copy; dup on-chip)
    xt_d = nc.dram_tensor("xt", [bpc, IN_DIM, IN_CAPS], bf16, kind="ExternalInput")
    # xs2[grp, par, i, slot] = sum_G X[b, (2kc+par)*64+G, i],  slot=j*16+kc
    xs2_d = nc.dram_tensor("xs2", [ng, 2, IN_DIM, 64], bf16, kind="ExternalInput")
    # wt2[d, g, h*64+k] = W[k, g*64+d]  (dup along free dim for z row-pack)
    wt2_d = nc.dram_tensor("wt2", [IN_DIM, 32, 128], bf16, kind="ExternalInput")
    # w2bd[par*64+k, g, par'*64+d] = W[k, g*64+d] if par==par' else 0
    w2bd_d = nc.dram_tensor("w2bd", [128, 32, 128], bf16, kind="ExternalInput")
    wsum_d = nc.dram_tensor("wsum", [IN_DIM, DIM], bf16, kind="ExternalInput")
    i128_d = nc.dram_tensor("i128", [128, 128], bf16, kind="ExternalInput")
    # e2f[q, p] = 1 if q%64 == p%64 else 0  (parity fold, full width)
    e2f_d = nc.dram_tensor("e2f", [128, 128], f32, kind="ExternalInput")
    # out viewed [b, kc, par, d]: row n = 2*kc+par is memory order [kc][par]
    out_d = nc.dram_tensor("out", [bpc, 16, 2, DIM], f32, kind="ExternalOutput")

    with tile.TileContext(nc) as tc:
        with (
            tc.tile_pool(name="const", bufs=1) as cpool,
            tc.tile_pool(name="inp", bufs=1) as ipool,
            tc.tile_pool(name="work", bufs=2) as wpool,
            tc.tile_pool(name="big", bufs=2) as bigpool,
            tc.tile_pool(name="ps_z", bufs=2, space="PSUM") as ps_z,
            tc.tile_pool(name="ps_db", bufs=2, space="PSUM") as ps_db,
            tc.tile_pool(name="ps_p", bufs=2, space="PSUM") as ps_p,
            tc.tile_pool(name="ps_o", bufs=1, space="PSUM") as ps_o,
            tc.tile_pool(name="ps_sm", bufs=1, space="PSUM") as ps_sm,
        ):
            # ---------- constant + input loads, priority-ordered, 4 queues ----
            wsum_t = cpool.tile([IN_DIM, DIM], bf16, tag="wsum")
            nc.sync.dma_start(wsum_t[:], wsum_d[:])
            i128_t = cpool.tile([128, 128], bf16, tag="i128")
            nc.scalar.dma_start(i128_t[:], i128_d[:])
            wt2_t = cpool.tile([IN_DIM, 32, 128], bf16, tag="wt2")
            nc.scalar.dma_start(wt2_t[:], wt2_d[:])
            e2f_t = cpool.tile([128, 128], f32, tag="e2f")
            nc.gpsimd.dma_start(e2f_t[:], e2f_d[:])
            # w2bd is first needed by m2s (late) — keep it off the gpsimd
            # queue so the xt2g dups there aren't stuck behind its 1MB, and
            # defer the issue until after the x loads (see ph_load).
            w2bd_t = cpool.tile([128, 32, 128], bf16, tag="w2bd")
            w2bd_loaded = []
            eps_t = cpool.tile([128, 1], f32, tag="eps")
            nc.gpsimd.memset(eps_t[:], EPS)
            one_t = cpool.tile([128, 1], f32, tag="one")
            nc.gpsimd.memset(one_t[:], 1.0 + EPS)

            # ---------- helpers ----------
            def squash(o_ps):
                """psum [64 slot, 2 par, 64 d] -> squashed f32 sbuf (same shape)."""
                o_sb = wpool.tile([64, 2, DIM], f32, tag="osb")
                nc.vector.tensor_copy(o_sb[:], o_ps[:])
                o2 = wpool.tile([64, 2, DIM], f32, tag="o2")
                s0 = wpool.tile([64, 2], f32, tag="s0")
                for par in range(2):
                    nc.scalar.activation(
                        o2[:, par, :], o_ps[:, par, :], Act.Square,
                        accum_out=s0[:, par : par + 1],
                    )
                # f = sqrt(s+eps)/(1+s+eps) = exp(0.5*ln(s+eps) - ln(1+s+eps))
                # (ln and exp share one act table set -> no table reloads,
                # unlike Sqrt which forces a 1.3us table swap around each exp)
                la = wpool.tile([64, 2], f32, tag="la")
                nc.scalar.activation(la[:], s0[:], Act.Ln, bias=eps_t[0:64])
                lb = wpool.tile([64, 2], f32, tag="lb")
                nc.scalar.activation(lb[:], s0[:], Act.Ln, bias=one_t[0:64])
                t = wpool.tile([64, 2], f32, tag="t")
                nc.vector.tensor_scalar_mul(t[:], la[:], 0.5)
                t2 = wpool.tile([64, 2], f32, tag="t2")
                nc.vector.tensor_sub(t2[:], t[:], lb[:])
                f = wpool.tile([64, 2], f32, tag="f")
                nc.scalar.activation(f[:], t2[:], Act.Exp)
                osq = wpool.tile([64, 2, DIM], f32, tag="osq")
                nc.vector.tensor_mul(
                    osq[:], o_sb[:], f[:, :, None].to_broadcast([64, 2, DIM])
                )
                return osq

            def transpose_o(o_sb):
                """o_sb f32 [64 slot, 2 par, 64 d] -> oT sbuf [64 d, 128 p'] bf16.

                PE transposes into the ps_sm bank (shared tag with the softmax
                fold tile); e2f's top-left quadrant doubles as f32 identity.
                """
                t_ps = ps_sm.tile([128, 128], f32, tag="sm")
                for par in range(2):
                    nc.tensor.transpose(
                        t_ps[0:64, par * 64 : (par + 1) * 64], o_sb[:, par, :],
                        e2f_t[0:64, 0:64],
                    )
                oT = wpool.tile([IN_DIM, 128], bf16, tag="oT")
                nc.vector.tensor_copy(oT[:], t_ps[0:64, :])
                return oT

            def zstep(oT):
                """oT [64,128] -> z2 sbuf [128 (h,k), 32 g, 128 p'] bf16 (dup halves)."""
                z2 = bigpool.tile([128, 32, 128], bf16, tag="z2")
                for gw in range(8):  # waves of 4 g
                    z_ps = ps_z.tile([128, 4, 128], f32, tag="z")
                    for j in range(4):
                        g = gw * 4 + j
                        nc.tensor.matmul(
                            z_ps[:, j, :], lhsT=wt2_t[:, g, :], rhs=oT[:],
                            start=True, stop=True,
                        )
                    if gw % 2 == 0:
                        nc.scalar.copy(z2[:, gw * 4 : gw * 4 + 4, :], z_ps[:])
                    else:
                        nc.vector.tensor_copy(z2[:, gw * 4 : gw * 4 + 4, :], z_ps[:])
                return z2

            def dbstep(z2, xt2g, b_prev, expb):
                """MM waves + per-wave (exp from PSUM | kc-reduce | b1 save).

                iter1 (b_prev None): saves the coefficients to SBUF as bf16.
                iter2: preloads b1 into PSUM via identity matmul and lets the
                db matmuls accumulate on top — no vector add needed.
                Returns (b1 or None, T [128, 32 g, GB] f32 exp-sums).
                """
                nb = None
                if b_prev is None:
                    nb = bigpool.tile([128, 32, 64], bf16, tag="b1")
                T = wpool.tile([128, 32, GB], f32, tag="T")
                for b in range(GB):  # one wave per batch: 16 slots
                    db_ps = ps_db.tile([128, 32, 16], f32, tag="db")
                    if b_prev is not None:
                        nc.tensor.matmul(
                            db_ps[:],
                            lhsT=i128_t[:],
                            rhs=b_prev[:, :, b * 16 : (b + 1) * 16],
                            start=True, stop=False, skip_group_check=True,
                        )
                    for kc in range(16):
                        slot = b * 16 + kc
                        for parity in range(2):
                            h = parity * 64
                            n = 2 * kc + parity
                            nc.tensor.matmul(
                                db_ps[h : h + 64, :, kc],
                                lhsT=xt2g[h : h + 64, b, n * 64 : (n + 1) * 64],
                                rhs=z2[h : h + 64, :, h + slot],
                                start=(b_prev is None), stop=True,
                                skip_group_check=(b_prev is not None),
                            )
                    eb = expb[:, :, b * 16 : (b + 1) * 16]
                    nc.scalar.activation(eb, db_ps[:], Act.Exp)
                    if b_prev is None:
                        nc.vector.tensor_copy(nb[:, :, b * 16 : (b + 1) * 16], db_ps[:])
                    nc.vector.tensor_reduce(T[:, :, b], eb, mybir.AxisListType.X, Alu.add)
                return nb, T

            def softmax_tail(T, expb):
                """T [128, 32, GB] -> c bf16 [128, 32 g, 64 slot] (4 b-waves)."""
                S_ps = ps_sm.tile([128, 32 * GB], f32, tag="sm")
                nc.tensor.matmul(
                    S_ps[:], lhsT=e2f_t[:], rhs=T[:].rearrange("p g b -> p (g b)"),
                    start=True, stop=True,
                )
                rs = wpool.tile([128, 32, GB], f32, tag="rs")
                nc.vector.reciprocal(rs[:].rearrange("p g b -> p (g b)"), S_ps[:])
                rs_bf = wpool.tile([128, 32, GB], bf16, tag="rsbf")
                nc.scalar.copy(rs_bf[:], rs[:])
                c_sb = bigpool.tile([128, 32, 64], bf16, tag="c")
                for b in range(GB):
                    nc.vector.tensor_mul(
                        c_sb[:, :, b * 16 : (b + 1) * 16],
                        expb[:, :, b * 16 : (b + 1) * 16],
                        rs_bf[:, :, b : b + 1].to_broadcast([128, 32, 16]),
                    )
                return c_sb

            def pstep(c_sb, x_g):
                """c [128,32,64] bf16 + x -> p_all sbuf [128 (par,k), 64 slot, 32 g]."""
                p_all = bigpool.tile([128, 64, 32], bf16, tag="pall")
                for b in range(GB):
                    p_ps = ps_p.tile([128, 16, 32], f32, tag="pw")
                    for kc in range(16):
                        slot = b * 16 + kc
                        for parity in range(2):
                            h = parity * 64
                            nc.tensor.matmul(
                                p_ps[h : h + 64, kc, :],
                                lhsT=x_g[h : h + 64, b, kc, :],
                                rhs=c_sb[h : h + 64, :, slot],
                                start=True, stop=True,
                            )
                    if b % 2 == 0:
                        nc.scalar.copy(p_all[:, b * 16 : (b + 1) * 16, :], p_ps[:])
                    else:
                        nc.vector.tensor_copy(p_all[:, b * 16 : (b + 1) * 16, :], p_ps[:])
                return p_all

            def mm2(p_all, o_ps):
                # o_ps [64 slot, 128 (par,d)] += sum_g p_all[:,:,g].T @ w2bd[:,g,:]
                for g in range(32):
                    nc.tensor.matmul(
                        o_ps[:],
                        lhsT=p_all[:, :, g],
                        rhs=w2bd_t[:, g, :],
                        start=(g == 0), stop=(g == 31),
                        skip_group_check=True,
                    )

            # ================= interleaved group emission =================
            st = [dict() for _ in range(ng)]

            def ph_load(g_):
                grp, s_ = g_, st[g_]
                xs_t = ipool.tile([IN_DIM, 2, 64], bf16, tag=f"xs{grp}")
                q_xs = nc.sync if grp == 0 else nc.scalar
                q_xs.dma_start(xs_t[:], xs2_d[grp].rearrange("par i s -> i par s"))
                s_["xs"] = xs_t
                # X^T bulk load (one HBM DMA) + on-chip dup to partitions 64:128
                xt2g = ipool.tile([128, GB, IN_CAPS], bf16, tag=f"xt{grp}")
                nc.sync.dma_start(
                    xt2g[0:64, :, :],
                    xt_d[grp * GB : (grp + 1) * GB].rearrange("b k r -> k b r"),
                )
                # split the on-chip dup so early db waves unblock sooner
                nc.gpsimd.dma_start(xt2g[64:128, 0:2, :], xt2g[0:64, 0:2, :])
                nc.gpsimd.dma_start(xt2g[64:128, 2:GB, :], xt2g[0:64, 2:GB, :])
                s_["xt2g"] = xt2g
                x_g = ipool.tile([128, GB, 16, IN_DIM], bf16, tag=f"x{grp}")
                nc.scalar.dma_start(
                    x_g[:],
                    x_d[grp * GB : (grp + 1) * GB].rearrange("b p k i -> p b k i"),
                )
                s_["x_g"] = x_g
                if grp == ng - 1 and not w2bd_loaded:
                    nc.scalar.dma_start(w2bd_t[:], w2bd_d[:])
                    w2bd_loaded.append(True)

            def ph_iter0(g_):
                s_ = st[g_]
                o_ps = ps_o.tile([64, 2, DIM], f32, tag="o")
                for par in range(2):
                    nc.tensor.matmul(
                        o_ps[:, par, :], lhsT=s_["xs"][:, par, :], rhs=wsum_t[:],
                        start=True, stop=True,
                    )
                s_["osq"] = squash(o_ps)
                s_["b"] = None

            def ph_tz(g_):
                s_ = st[g_]
                oT = transpose_o(s_["osq"])
                s_["z2"] = zstep(oT)

            def ph_db(g_):
                s_ = st[g_]
                expb = bigpool.tile([128, 32, 64], bf16, tag="expb")
                nb, s_["T"] = dbstep(s_["z2"], s_["xt2g"], s_["b"], expb)
                if s_["b"] is None:
                    s_["b"] = nb
                s_["expb"] = expb

            def ph_smp(g_):
                s_ = st[g_]
                c_sb = softmax_tail(s_["T"], s_["expb"])
                s_["pall"] = pstep(c_sb, s_["x_g"])

            def ph_m2s(g_):
                s_ = st[g_]
                o_ps = ps_o.tile([64, 2, DIM], f32, tag="o")
                mm2(s_["pall"], o_ps[:])
                s_["osq"] = squash(o_ps)

            def ph_out(g_):
                grp, s_ = g_, st[g_]
                nc.gpsimd.dma_start(
                    out_d[grp * GB : (grp + 1) * GB].rearrange(
                        "b kc par d -> (b kc) par d"
                    ),
                    s_["osq"][:],
                )

            phases = [ph_load, ph_iter0, ph_tz, ph_db, ph_smp, ph_m2s,
                      ph_tz, ph_db, ph_smp, ph_m2s, ph_out]
            OFFSET = 1
            for k in range(len(phases) + OFFSET * (ng - 1)):
                for grp in range(ng):
                    kk = k - OFFSET * grp
                    if 0 <= kk < len(phases):
                        phases[kk](grp)

    nc.compile()
    return nc


def _get_nc():
    if "nc" not in _CACHE:
        _CACHE["nc"] = _build_nc()
    return _CACHE["nc"]


def _prep_host_small(inputs, kern):
    """Host-side input prep; inputs [Bn, 2048, 64] with Bn a multiple of GB."""
    import ml_dtypes

    bf = ml_dtypes.bfloat16
    Bn = inputs.shape[0]
    ng = Bn // GB
    X = np.ascontiguousarray(inputs, dtype=np.float32)
    W = np.ascontiguousarray(kern.reshape(IN_DIM, NUM * DIM), dtype=np.float32)

    # x[b, parity*64+G, kc, i] = X[b, (2kc+parity)*64+G, i]
    xr = X.reshape(Bn, 16, 2, 64, IN_DIM)          # [b, kc, parity, G, i]
    x_h = np.ascontiguousarray(xr.transpose(0, 2, 3, 1, 4).reshape(Bn, 128, 16, IN_DIM))
    xt_h = np.ascontiguousarray(X.transpose(0, 2, 1))  # [b, k, r]
    # xs2[grp, par, i, j*16+kc] = sum_G X[grp*GB+j, (2kc+par)*64+G, i]
    xsum = xr.sum(axis=3)                           # [b, kc, parity, i]
    xs2_h = np.ascontiguousarray(
        xsum.reshape(ng, GB, 16, 2, IN_DIM).transpose(0, 3, 4, 1, 2)
        .reshape(ng, 2, IN_DIM, 64)
    )
    Wr = W.reshape(IN_DIM, 32, 64)                  # [k, g, d]
    wt_h = Wr.transpose(2, 1, 0)                    # [d, g, k]
    wt2_h = np.ascontiguousarray(np.concatenate([wt_h, wt_h], axis=2))  # [64,32,128]
    w2bd_h = np.zeros((2, IN_DIM, 32, 2, 64), np.float32)
    for par in range(2):
        w2bd_h[par, :, :, par, :] = Wr
    w2bd_h = np.ascontiguousarray(w2bd_h.reshape(128, 32, 128))
    wsum_h = np.ascontiguousarray(Wr.sum(axis=1) / 32.0)
    i128_h = np.eye(128, dtype=np.float32)
    e2f_h = np.ascontiguousarray(np.tile(np.eye(64, dtype=np.float32), (2, 2)))
    return (
        x_h.astype(bf), xt_h.astype(bf), xs2_h.astype(bf),
        wt2_h.astype(bf), w2bd_h.astype(bf), wsum_h.astype(bf), i128_h.astype(bf),
        e2f_h,
    )


def _make_in_maps(inputs, kern):
    x_h, xt_h, xs2_h, wt2_h, w2bd_h, wsum_h, i128_h, e2f_h = _prep_host_small(
        np.asarray(inputs), np.asarray(kern)
    )
    in_maps = []
    for c in range(N_CORES):
        sl = slice(c * BPC, (c + 1) * BPC)
        gsl = slice(c * NG, (c + 1) * NG)
        in_maps.append(
            {
                "x": x_h[sl], "xt": xt_h[sl], "xs2": xs2_h[gsl],
                "wt2": wt2_h, "w2bd": w2bd_h, "wsum": wsum_h, "i128": i128_h,
                "e2f": e2f_h,
            }
        )
    return in_maps


def kernel(inputs, kernel, num_capsule=NUM, dim_capsule=DIM, routings=3, **_):
    from concourse.bass_utils import run_bass_kernel_spmd

    assert int(num_capsule) == NUM and int(dim_capsule) == DIM and int(routings) == 3
    nc = _get_nc()
    in_maps = _make_in_maps(inputs, kernel)
    res = run_bass_kernel_spmd(nc, in_maps, core_ids=list(range(N_CORES)))
    out = np.concatenate([res.results[c]["out"] for c in range(N_CORES)], axis=0)
    return out.reshape(B, NUM, DIM).astype(np.float32)


# revision 37
# speedup vs baseline: 1.0696x; 1.0696x over previous
"""Capsule-routing kernel v3 — batch-merged, parity-packed, DMA/engine optimized.

Per core: 8 batches in 2 groups of 4, processed in lockstep so that matmuls
merge across batches and vector/scalar ops run at full [128, *] width.

Index conventions (per group of GB=4 batches):
  capsule n = 2*kc + parity   (kc in [0,16), parity in {0,1})
  slot(b, kc) = b*16 + kc     in [0, 64)
  p'(b, n)  = parity*64 + slot  -> z column order (parity-major)
  b/c layout: [128 (parity*64+G), 32 g, 64 slot]
  o layout:   [64 slot, 2 parity, 64 d]   (enables single out-DMA per group)
Row-packed MM pairs: even capsules use partitions 0:64, odd 64:128, running
concurrently in distinct PE quadrants (bf16).

v3 changes vs v2:
  - DMA: single-copy X^T + on-chip dup, per-group bulk loads, 4 issue queues,
    priority ordering (first matmul ~3us instead of ~17us), 1 out-DMA/group.
  - zstep: one MM per g via duplicated-free-dim stationary wt2 (32 vs 64 MMs).
  - mm2: one MM per g via block-diagonal W (32 vs 64 MMs), new o layout.
  - softmax: full-width fold matrix e2f (drops the broadcast matmul), exp
    reads PSUM directly (overlaps the b-coefficient copy), bf16 exp/cmul,
    kc-reduction on gpsimd.
"""

import numpy as np

B, IN_CAPS, IN_DIM = 64, 2048, 64
NUM, DIM = 32, 64
N_CORES = 8
BPC = B // N_CORES  # 8 batches per core
GB = 4              # batches per merged group
NG = BPC // GB      # 2 groups
EPS = 1e-7

_CACHE = {}


def _build_nc(bpc=BPC):
    import concourse.bacc as bacc
    import concourse.tile as tile
    from concourse import mybir

    f32 = mybir.dt.float32
    bf16 = mybir.dt.bfloat16
    Act = mybir.ActivationFunctionType
    Alu = mybir.AluOpType

    ng = bpc // GB
    nc = bacc.Bacc("TRN2", target_bir_lowering=False, debug=False, num_devices=N_CORES)

    # ---- DRAM I/O (per-core shapes) ----
    # x[b, parity*64+G, kc, i] = X[b, (2kc+parity)*64+G, i]
    x_d = nc.dram_tensor("x", [bpc, 128, 16, IN_DIM], bf16, kind="ExternalInput")
    # xt[b, k, r] = X[b, r, k]   (single copy; dup on-chip)
    xt_d = nc.dram_tensor("xt", [bpc, IN_DIM, IN_CAPS], bf16, kind="ExternalInput")
    # xs2[grp, par, i, slot] = sum_G X[b, (2kc+par)*64+G, i],  slot=j*16+kc
    xs2_d = nc.dram_tensor("xs2", [ng, 2, IN_DIM, 64], bf16, kind="ExternalInput")
    # wt2[d, g, h*64+k] = W[k, g*64+d]  (dup along free dim for z row-pack)
    wt2_d = nc.dram_tensor("wt2", [IN_DIM, 32, 128], bf16, kind="ExternalInput")
    # w2bd[par*64+k, g, par'*64+d] = W[k, g*64+d] if par==par' else 0
    w2bd_d = nc.dram_tensor("w2bd", [128, 32, 128], bf16, kind="ExternalInput")
    wsum_d = nc.dram_tensor("wsum", [IN_DIM, DIM], bf16, kind="ExternalInput")
    i128_d = nc.dram_tensor("i128", [128, 128], bf16, kind="ExternalInput")
    # e2f[q, p] = 1 if q%64 == p%64 else 0  (parity fold, full width)
    e2f_d = nc.dram_tensor("e2f", [128, 128], f32, kind="ExternalInput")
    # out viewed [b, kc, par, d]: row n = 2*kc+par is memory order [kc][par]
    out_d = nc.dram_tensor("out", [bpc, 16, 2, DIM], f32, kind="ExternalOutput")

    with tile.TileContext(nc) as tc:
        with (
            tc.tile_pool(name="const", bufs=1) as cpool,
            tc.tile_pool(name="inp", bufs=1) as ipool,
            tc.tile_pool(name="work", bufs=2) as wpool,
            tc.tile_pool(name="big", bufs=2) as bigpool,
            tc.tile_pool(name="ps_z", bufs=2, space="PSUM") as ps_z,
            tc.tile_pool(name="ps_db", bufs=2, space="PSUM") as ps_db,
            tc.tile_pool(name="ps_p", bufs=2, space="PSUM") as ps_p,
            tc.tile_pool(name="ps_o", bufs=1, space="PSUM") as ps_o,
            tc.tile_pool(name="ps_sm", bufs=1, space="PSUM") as ps_sm,
        ):
            # ---------- constant + input loads, priority-ordered, 4 queues ----
            wsum_t = cpool.tile([IN_DIM, DIM], bf16, tag="wsum")
            nc.sync.dma_start(wsum_t[:], wsum_d[:])
            i128_t = cpool.tile([128, 128], bf16, tag="i128")
            nc.scalar.dma_start(i128_t[:], i128_d[:])
            wt2_t = cpool.tile([IN_DIM, 32, 128], bf16, tag="wt2")
            nc.scalar.dma_start(wt2_t[:], wt2_d[:])
            e2f_t = cpool.tile([128, 128], f32, tag="e2f")
            nc.gpsimd.dma_start(e2f_t[:], e2f_d[:])
            w2bd_t = cpool.tile([128, 32, 128], bf16, tag="w2bd")
            nc.gpsimd.dma_start(w2bd_t[:], w2bd_d[:])
            eps_t = cpool.tile([128, 1], f32, tag="eps")
            nc.gpsimd.memset(eps_t[:], EPS)
            one_t = cpool.tile([128, 1], f32, tag="one")
            nc.gpsimd.memset(one_t[:], 1.0 + EPS)

            # ---------- helpers ----------
            def squash(o_ps):
                """psum [64 slot, 2 par, 64 d] -> squashed f32 sbuf (same shape)."""
                o_sb = wpool.tile([64, 2, DIM], f32, tag="osb")
                nc.vector.tensor_copy(o_sb[:], o_ps[:])
                o2 = wpool.tile([64, 2, DIM], f32, tag="o2")
                s0 = wpool.tile([64, 2], f32, tag="s0")
                for par in range(2):
                    nc.scalar.activation(
                        o2[:, par, :], o_ps[:, par, :], Act.Square,
                        accum_out=s0[:, par : par + 1],
                    )
                u = wpool.tile([64, 2], f32, tag="u")
                nc.scalar.activation(u[:], s0[:], Act.Sqrt, bias=eps_t[0:64])
                v = wpool.tile([64, 2], f32, tag="v")
                nc.vector.tensor_scalar_add(v[:], s0[:], 1.0 + EPS)
                rv = wpool.tile([64, 2], f32, tag="rv")
                nc.vector.reciprocal(rv[:], v[:])
                f = wpool.tile([64, 2], f32, tag="f")
                nc.vector.tensor_mul(f[:], u[:], rv[:])
                osq = wpool.tile([64, 2, DIM], f32, tag="osq")
                nc.vector.tensor_mul(
                    osq[:], o_sb[:], f[:, :, None].to_broadcast([64, 2, DIM])
                )
                return osq

            def transpose_o(o_sb):
                """o_sb f32 [64 slot, 2 par, 64 d] -> oT sbuf [64 d, 128 p'] bf16.

                PE transposes into the ps_sm bank (shared tag with the softmax
                fold tile); e2f's top-left quadrant doubles as f32 identity.
                """
                t_ps = ps_sm.tile([128, 128], f32, tag="sm")
                for par in range(2):
                    nc.tensor.transpose(
                        t_ps[0:64, par * 64 : (par + 1) * 64], o_sb[:, par, :],
                        e2f_t[0:64, 0:64],
                    )
                oT = wpool.tile([IN_DIM, 128], bf16, tag="oT")
                nc.vector.tensor_copy(oT[:], t_ps[0:64, :])
                return oT

            def zstep(oT):
                """oT [64,128] -> z2 sbuf [128 (h,k), 32 g, 128 p'] bf16 (dup halves)."""
                z2 = bigpool.tile([128, 32, 128], bf16, tag="z2")
                for gw in range(8):  # waves of 4 g
                    z_ps = ps_z.tile([128, 4, 128], f32, tag="z")
                    for j in range(4):
                        g = gw * 4 + j
                        nc.tensor.matmul(
                            z_ps[:, j, :], lhsT=wt2_t[:, g, :], rhs=oT[:],
                            start=True, stop=True,
                        )
                    if gw % 2 == 0:
                        nc.scalar.copy(z2[:, gw * 4 : gw * 4 + 4, :], z_ps[:])
                    else:
                        nc.vector.tensor_copy(z2[:, gw * 4 : gw * 4 + 4, :], z_ps[:])
                return z2

            def dbstep(z2, xt2g, b_prev, expb):
                """MM waves + per-wave (exp from PSUM | kc-reduce | b1 save).

                iter1 (b_prev None): saves the coefficients to SBUF as bf16.
                iter2: preloads b1 into PSUM via identity matmul and lets the
                db matmuls accumulate on top — no vector add needed.
                Returns (b1 or None, T [128, 32 g, GB] f32 exp-sums).
                """
                nb = None
                if b_prev is None:
                    nb = bigpool.tile([128, 32, 64], bf16, tag="b1")
                T = wpool.tile([128, 32, GB], f32, tag="T")
                for b in range(GB):  # one wave per batch: 16 slots
                    db_ps = ps_db.tile([128, 32, 16], f32, tag="db")
                    if b_prev is not None:
                        nc.tensor.matmul(
                            db_ps[:],
                            lhsT=i128_t[:],
                            rhs=b_prev[:, :, b * 16 : (b + 1) * 16],
                            start=True, stop=False, skip_group_check=True,
                        )
                    for kc in range(16):
                        slot = b * 16 + kc
                        for parity in range(2):
                            h = parity * 64
                            n = 2 * kc + parity
                            nc.tensor.matmul(
                                db_ps[h : h + 64, :, kc],
                                lhsT=xt2g[h : h + 64, b, n * 64 : (n + 1) * 64],
                                rhs=z2[h : h + 64, :, h + slot],
                                start=(b_prev is None), stop=True,
                                skip_group_check=(b_prev is not None),
                            )
                    eb = expb[:, :, b * 16 : (b + 1) * 16]
                    nc.scalar.activation(eb, db_ps[:], Act.Exp)
                    if b_prev is None:
                        nc.vector.tensor_copy(nb[:, :, b * 16 : (b + 1) * 16], db_ps[:])
                    nc.vector.tensor_reduce(T[:, :, b], eb, mybir.AxisListType.X, Alu.add)
                return nb, T

            def softmax_tail(T, expb):
                """T [128, 32, GB] -> c bf16 [128, 32 g, 64 slot] (4 b-waves)."""
                S_ps = ps_sm.tile([128, 32 * GB], f32, tag="sm")
                nc.tensor.matmul(
                    S_ps[:], lhsT=e2f_t[:], rhs=T[:].rearrange("p g b -> p (g b)"),
                    start=True, stop=True,
                )
                rs = wpool.tile([128, 32, GB], f32, tag="rs")
                nc.vector.reciprocal(rs[:].rearrange("p g b -> p (g b)"), S_ps[:])
                rs_bf = wpool.tile([128, 32, GB], bf16, tag="rsbf")
                nc.scalar.copy(rs_bf[:], rs[:])
                c_sb = bigpool.tile([128, 32, 64], bf16, tag="c")
                for b in range(GB):
                    nc.vector.tensor_mul(
                        c_sb[:, :, b * 16 : (b + 1) * 16],
                        expb[:, :, b * 16 : (b + 1) * 16],
                        rs_bf[:, :, b : b + 1].to_broadcast([128, 32, 16]),
                    )
                return c_sb

            def pstep(c_sb, x_g):
                """c [128,32,64] bf16 + x -> p_all sbuf [128 (par,k), 64 slot, 32 g]."""
                p_all = bigpool.tile([128, 64, 32], bf16, tag="pall")
                for b in range(GB):
                    p_ps = ps_p.tile([128, 16, 32], f32, tag="pw")
                    for kc in range(16):
                        slot = b * 16 + kc
                        for parity in range(2):
                            h = parity * 64
                            nc.tensor.matmul(
                                p_ps[h : h + 64, kc, :],
                                lhsT=x_g[h : h + 64, b, kc, :],
                                rhs=c_sb[h : h + 64, :, slot],
                                start=True, stop=True,
                            )
                    if b % 2 == 0:
                        nc.scalar.copy(p_all[:, b * 16 : (b + 1) * 16, :], p_ps[:])
                    else:
                        nc.vector.tensor_copy(p_all[:, b * 16 : (b + 1) * 16, :], p_ps[:])
                return p_all

            def mm2(p_all, o_ps):
                # o_ps [64 slot, 128 (par,d)] += sum_g p_all[:,:,g].T @ w2bd[:,g,:]
                for g in range(32):
                    nc.tensor.matmul(
                        o_ps[:],
                        lhsT=p_all[:, :, g],
                        rhs=w2bd_t[:, g, :],
                        start=(g == 0), stop=(g == 31),
                        skip_group_check=True,
                    )

            # ================= interleaved group emission =================
            st = [dict() for _ in range(ng)]

            def ph_load(g_):
                grp, s_ = g_, st[g_]
                xs_t = ipool.tile([IN_DIM, 2, 64], bf16, tag=f"xs{grp}")
                q_xs = nc.sync if grp == 0 else nc.scalar
                q_xs.dma_start(xs_t[:], xs2_d[grp].rearrange("par i s -> i par s"))
                s_["xs"] = xs_t
                # X^T bulk load (one HBM DMA) + on-chip dup to partitions 64:128
                xt2g = ipool.tile([128, GB, IN_CAPS], bf16, tag=f"xt{grp}")
                nc.sync.dma_start(
                    xt2g[0:64, :, :],
                    xt_d[grp * GB : (grp + 1) * GB].rearrange("b k r -> k b r"),
                )
                nc.gpsimd.dma_start(xt2g[64:128, :, :], xt2g[0:64, :, :])
                s_["xt2g"] = xt2g
                x_g = ipool.tile([128, GB, 16, IN_DIM], bf16, tag=f"x{grp}")
                nc.scalar.dma_start(
                    x_g[:],
                    x_d[grp * GB : (grp + 1) * GB].rearrange("b p k i -> p b k i"),
                )
                s_["x_g"] = x_g

            def ph_iter0(g_):
                s_ = st[g_]
                o_ps = ps_o.tile([64, 2, DIM], f32, tag="o")
                for par in range(2):
                    nc.tensor.matmul(
                        o_ps[:, par, :], lhsT=s_["xs"][:, par, :], rhs=wsum_t[:],
                        start=True, stop=True,
                    )
                s_["osq"] = squash(o_ps)
                s_["b"] = None

            def ph_tz(g_):
                s_ = st[g_]
                oT = transpose_o(s_["osq"])
                s_["z2"] = zstep(oT)

            def ph_db(g_):
                s_ = st[g_]
                expb = bigpool.tile([128, 32, 64], bf16, tag="expb")
                nb, s_["T"] = dbstep(s_["z2"], s_["xt2g"], s_["b"], expb)
                if s_["b"] is None:
                    s_["b"] = nb
                s_["expb"] = expb

            def ph_smp(g_):
                s_ = st[g_]
                c_sb = softmax_tail(s_["T"], s_["expb"])
                s_["pall"] = pstep(c_sb, s_["x_g"])

            def ph_m2s(g_):
                s_ = st[g_]
                o_ps = ps_o.tile([64, 2, DIM], f32, tag="o")
                mm2(s_["pall"], o_ps[:])
                s_["osq"] = squash(o_ps)

            def ph_out(g_):
                grp, s_ = g_, st[g_]
                nc.gpsimd.dma_start(
                    out_d[grp * GB : (grp + 1) * GB].rearrange(
                        "b kc par d -> (b kc) par d"
                    ),
                    s_["osq"][:],
                )

            phases = [ph_load, ph_iter0, ph_tz, ph_db, ph_smp, ph_m2s,
                      ph_tz, ph_db, ph_smp, ph_m2s, ph_out]
            OFFSET = 1
            for k in range(len(phases) + OFFSET * (ng - 1)):
                for grp in range(ng):
                    kk = k - OFFSET * grp
                    if 0 <= kk < len(phases):
                        phases[kk](grp)

    nc.compile()
    return nc


def _get_nc():
    if "nc" not in _CACHE:
        _CACHE["nc"] = _build_nc()
    return _CACHE["nc"]


def _prep_host_small(inputs, kern):
    """Host-side input prep; inputs [Bn, 2048, 64] with Bn a multiple of GB."""
    import ml_dtypes

    bf = ml_dtypes.bfloat16
    Bn = inputs.shape[0]
    ng = Bn // GB
    X = np.ascontiguousarray(inputs, dtype=np.float32)
    W = np.ascontiguousarray(kern.reshape(IN_DIM, NUM * DIM), dtype=np.float32)

    # x[b, parity*64+G, kc, i] = X[b, (2kc+parity)*64+G, i]
    xr = X.reshape(Bn, 16, 2, 64, IN_DIM)          # [b, kc, parity, G, i]
    x_h = np.ascontiguousarray(xr.transpose(0, 2, 3, 1, 4).reshape(Bn, 128, 16, IN_DIM))
    xt_h = np.ascontiguousarray(X.transpose(0, 2, 1))  # [b, k, r]
    # xs2[grp, par, i, j*16+kc] = sum_G X[grp*GB+j, (2kc+par)*64+G, i]
    xsum = xr.sum(axis=3)                           # [b, kc, parity, i]
    xs2_h = np.ascontiguousarray(
        xsum.reshape(ng, GB, 16, 2, IN_DIM).transpose(0, 3, 4, 1, 2)
        .reshape(ng, 2, IN_DIM, 64)
    )
    Wr = W.reshape(IN_DIM, 32, 64)                  # [k, g, d]
    wt_h = Wr.transpose(2, 1, 0)                    # [d, g, k]
    wt2_h = np.ascontiguousarray(np.concatenate([wt_h, wt_h], axis=2))  # [64,32,128]
    w2bd_h = np.zeros((2, IN_DIM, 32, 2, 64), np.float32)
    for par in range(2):
        w2bd_h[par, :, :, par, :] = Wr
    w2bd_h = np.ascontiguousarray(w2bd_h.reshape(128, 32, 128))
    wsum_h = np.ascontiguousarray(Wr.sum(axis=1) / 32.0)
    i128_h = np.eye(128, dtype=np.float32)
    e2f_h = np.ascontiguousarray(np.tile(np.eye(64, dtype=np.float32), (2, 2)))
    return (
        x_h.astype(bf), xt_h.astype(bf), xs2_h.astype(bf),
        wt2_h.astype(bf), w2bd_h.astype(bf), wsum_h.astype(bf), i128_h.astype(bf),
        e2f_h,
    )


def _make_in_maps(inputs, kern):
    x_h, xt_h, xs2_h, wt2_h, w2bd_h, wsum_h, i128_h, e2f_h = _prep_host_small(
        np.asarray(inputs), np.asarray(kern)
    )
    in_maps = []
    for c in range(N_CORES):
        sl = slice(c * BPC, (c + 1) * BPC)
        gsl = slice(c * NG, (c + 1) * NG)
        in_maps.append(
            {
                "x": x_h[sl], "xt": xt_h[sl], "xs2": xs2_h[gsl],
                "wt2": wt2_h, "w2bd": w2bd_h, "wsum": wsum_h, "i128": i128_h,
                "e2f": e2f_h,
            }
        )
    return in_maps


def kernel(inputs, kernel, num_capsule=NUM, dim_capsule=DIM, routings=3, **_):
    from concourse.bass_utils import run_bass_kernel_spmd

    assert int(num_capsule) == NUM and int(dim_capsule) == DIM and int(routings) == 3
    nc = _get_nc()
    in_maps = _make_in_maps(inputs, kernel)
    res = run_bass_kernel_spmd(nc, in_maps, core_ids=list(range(N_CORES)))
    out = np.concatenate([res.results[c]["out"] for c in range(N_CORES)], axis=0)
    return out.reshape(B, NUM, DIM).astype(np.float32)
